# revision 27
# baseline (speedup 1.0000x reference)
"""CFConv (SchNet continuous-filter convolution) on 8 TRN2 NeuronCores.

Strategy (v4, ~448us vs 1226us v1 baseline): the per-edge source gather is
done on the HOST as a pure permutation -- h rows are staged into edge order
(feature-major bf16) and streamed sequentially, so the device never runs a
dynamic gather (the v1 GpSimd SWDGE gather was 86% of the kernel).  lin1 is
applied per edge on the TensorEngine.  Destination blocks of 128 nodes are
load-balanced across the 8 cores by sorting all 392 blocks by edge count
and dealing rank 8j..8j+7 to position j of cores 0..7, so one SPMD program
(per-position chunk counts k_list) fits every core with ~4% padding.  The
scatter-add is a sequence of one-hot matmuls accumulating in PSUM; each
block's group STARTS with the lin1 matmul of the block's own nodes, so
h2 = h1 + agg falls out of PSUM for free.  The filter MLP runs
feature-major with stationary weights at 1024-edge width; softplus is Exp
then Ln(x+1) from the single exp+ln activation table (softplus-LOG2 via
Ln(0.5x+0.5)); the cosine cutoff is folded into the one-hot scatter values
on the host exactly as the reference factorizes it.

The loop is software-pipelined over 1024-edge pairs with a 2-pair skew
between the layer-1 acts of pair p and the layer-2 acts of pair p-2, which
keeps the scalar engine (the bottleneck: 4 activation passes per edge, its
streaming floor) >90% busy with zero-gap issue.  PSUM discipline: a bank
must never host two concurrently-open matmul accumulation groups, so the
two alternating agg accumulators (long-open groups) own one bank, all
single-shot outputs (transposed messages, lin2 out, transposed h2) share
another, and the deferred per-block epilogue is flushed at loop tails so
it never head-blocks the PE queue.
"""
import sys

sys.path.insert(0, "/opt/trn_rl_repo")

import numpy as np
import ml_dtypes

import concourse.bass as bass
import concourse.mybir as mybir
import concourse.tile as tile
from concourse import bacc
from concourse import bass_utils
from concourse import hw_specs
import concourse.bacc as bacc_mod
from concourse.masks import make_identity

BF16 = ml_dtypes.bfloat16
F32 = np.float32
LOG2 = float(np.log(2.0))
CUTOFF = 10.0
PI = float(np.pi)

N_NODES = 50000
N_EDGES = 800000
CH = 128
NG = 50
NCORES = 8
P = 128

dt = mybir.dt

# Route Exp/Ln to the single table that holds both, so the scalar engine
# never reloads activation tables mid-kernel.  Table ids are positional, so
# preserve dict order and only edit membership.
_orig_tables = hw_specs.get_activation_tables


def _patched_tables(arch):
    t = _orig_tables(arch)
    for name, funcs in t.items():
        if name != "natural_log_exp_and_others":
            funcs.discard(mybir.ActivationFunctionType.Exp)
            funcs.discard(mybir.ActivationFunctionType.Ln)
    return t


bacc_mod.get_activation_tables = _patched_tables


def _ceil_div(a, b):
    return -(-a // b)


def build_program(n_chp, k_list, nblk, num_devices=NCORES):
    nc = bacc.Bacc(
        "TRN2",
        target_bir_lowering=False,
        debug=False,
        enable_asserts=False,
        num_devices=num_devices,
    )

    ne_pad = n_chp * P
    n_sup = n_chp // 4
    n_ch = sum(k_list)

    # chunk -> (block position, first?, last?)
    sched = []
    for j, kb in enumerate(k_list):
        for i in range(kb):
            sched.append((j, i == 0, i == kb - 1))
    assert len(sched) == n_ch

    # ---- DRAM I/O ----
    he_t = nc.dram_tensor("he_t", [P, ne_pad], dt.bfloat16, kind="ExternalInput")
    ea_t = nc.dram_tensor("ea_t", [NG, ne_pad], dt.bfloat16, kind="ExternalInput")
    s_t = nc.dram_tensor("s_t", [P, ne_pad], dt.bfloat16, kind="ExternalInput")
    hown_t = nc.dram_tensor("hown_t", [P, nblk * P], dt.bfloat16, kind="ExternalInput")
    w1t = nc.dram_tensor("w1t", [NG, CH], dt.bfloat16, kind="ExternalInput")
    w2t = nc.dram_tensor("w2t", [CH, CH], dt.bfloat16, kind="ExternalInput")
    lin1wt = nc.dram_tensor("lin1wt", [CH, CH], dt.bfloat16, kind="ExternalInput")
    lin2wt = nc.dram_tensor("lin2wt", [CH, CH], dt.bfloat16, kind="ExternalInput")
    b1 = nc.dram_tensor("b1", [P, 1], dt.float32, kind="ExternalInput")
    b2p = nc.dram_tensor("b2p", [P, 1], dt.float32, kind="ExternalInput")
    l2b = nc.dram_tensor("l2b", [P, 1], dt.float32, kind="ExternalInput")

    out_t = nc.dram_tensor("out_t", [P, nblk * P], dt.float32, kind="ExternalOutput")

    with tile.TileContext(nc) as tc:
        with tc.tile_pool(name="cpool", bufs=1) as cpool:
            # ---- constants ----
            w1t_sb = cpool.tile([NG, CH], dt.bfloat16, tag="w1t")
            nc.sync.dma_start(out=w1t_sb[:], in_=w1t.ap())
            b1_sb = cpool.tile([P, 1], dt.float32, tag="b1")
            nc.sync.dma_start(out=b1_sb[:], in_=b1.ap())
            # remaining constants are loaded after the first stream loads
            # (below) so the first MLP pair starts as early as possible
            w2t_sb = cpool.tile([CH, CH], dt.bfloat16, tag="w2t")
            lin1wt_sb = cpool.tile([CH, CH], dt.bfloat16, tag="lin1wt")
            lin2wt_sb = cpool.tile([CH, CH], dt.bfloat16, tag="lin2wt")
            b2p_sb = cpool.tile([P, 1], dt.float32, tag="b2p")
            l2b_sb = cpool.tile([P, 1], dt.float32, tag="l2b")
            half_sb = cpool.tile([P, 1], dt.float32, tag="half")
            nc.gpsimd.memset(half_sb[:], 0.5)
            ident_sb = cpool.tile([P, P], dt.bfloat16, tag="ident")
            make_identity(nc, ident_sb[:])
            hown_sb = cpool.tile([P, nblk * P], dt.bfloat16, tag="hown")
            # (its DMA is issued after the first stream loads, below, so the
            # first MLP pair is not stuck behind 1.6 MB of skip-connection
            # data on the sync queue)

            # ---- edge pipeline (software-pipelined over 1024-edge pairs)
            # loop p issues: loads(p), acts-L2(p-2), mlp1(p), acts-L1(p),
            # lin1(p-2), mult(p-2), transpose+scatter(p-2), mlp2(p),
            # deferred epilogues.  The scalar stream is then gap-free and
            # the PE never head-blocks on a fresh dependency.
            with (
                tc.tile_pool(name="pea", bufs=4) as pea,
                tc.tile_pool(name="pst", bufs=4) as pst,
                tc.tile_pool(name="phe", bufs=4) as phe,
                tc.tile_pool(name="px", bufs=3) as px,
                tc.tile_pool(name="pw", bufs=3) as pw,
                tc.tile_pool(name="pg", bufs=3) as pg,
                tc.tile_pool(name="pep", bufs=3) as pep,
                tc.tile_pool(name="psxw", bufs=2, space="PSUM") as psxw,
                tc.tile_pool(name="psh", bufs=2, space="PSUM") as psh,
                tc.tile_pool(name="psm", bufs=1, space="PSUM") as psm,
                tc.tile_pool(name="psao", bufs=1, space="PSUM") as psao,
            ):
                n_pair = n_chp // 8
                st = {}       # per-pair live tiles
                cst = {}      # per-couple activation tiles
                agg_state = [None]  # current agg region across scatter calls
                pending = []  # deferred block epilogues

                # Persistent PSUM banks.  A bank must never host two
                # concurrently-open matmul accumulation groups, so the two
                # alternating agg accumulators (long-lived groups) get a bank
                # to themselves, and everything single-shot (transposed
                # messages, lin2 output, transposed h2) shares the other.
                ao_ps = psao.tile([P, 256], dt.float32, tag="aggo")
                mo_ps = psm.tile([P, 448], dt.float32, tag="mo")

                def stage_load(p):
                    es = p * 1024
                    ea_sb = pea.tile([NG, 1024], dt.bfloat16, tag="ea")
                    nc.sync.dma_start(out=ea_sb[:], in_=ea_t.ap()[:, es : es + 1024])
                    s_sb = pst.tile([P, 1024], dt.bfloat16, tag="s_sel")
                    nc.sync.dma_start(out=s_sb[:], in_=s_t.ap()[:, es : es + 1024])
                    he_sb = phe.tile([P, 1024], dt.bfloat16, tag="he")
                    nc.sync.dma_start(out=he_sb[:], in_=he_t.ap()[:, es : es + 1024])
                    st[p] = dict(ea=ea_sb, s=s_sb, he=he_sb)

                def stage_lin1(p):
                    d = st[p]
                    d["h1g"] = []
                    for h in range(2):
                        h1g_ps = psh.tile([P, 512], dt.float32, tag="h1g")
                        nc.tensor.matmul(
                            out=h1g_ps[:], lhsT=lin1wt_sb[:],
                            rhs=d["he"][:, h * 512 : (h + 1) * 512],
                            start=True, stop=True,
                        )
                        d["h1g"].append(h1g_ps)

                def stage_mlp1(p):
                    d = st[p]
                    xw_ps = psxw.tile([P, 1024], dt.float32, tag="xw_ps")
                    for h in range(2):
                        nc.tensor.matmul(
                            out=xw_ps[:, h * 512 : (h + 1) * 512], lhsT=w1t_sb[:],
                            rhs=d["ea"][:, h * 512 : (h + 1) * 512],
                            start=True, stop=True,
                        )
                    d["xw"] = xw_ps

                def stage_exp1(p):
                    # e1 for a pair couple lives in one [P, 2048] tile so the
                    # following Ln runs once per couple at full width
                    d = st[p]
                    if p % 2 == 0:
                        e1c = px.tile([P, 2048], dt.float32, tag="e1")
                        cst[p // 2] = dict(e1=e1c)
                    e1c = cst[p // 2]["e1"]
                    off = (p % 2) * 1024
                    nc.scalar.activation(
                        out=e1c[:, off : off + 1024], in_=d["xw"][:],
                        func=mybir.ActivationFunctionType.Exp,
                        bias=b1_sb[:, 0:1],
                    )

                def stage_ln1(q):
                    e1c = cst[q]["e1"]
                    x_sb = px.tile([P, 2048], dt.bfloat16, tag="x_sb")
                    nc.scalar.activation(
                        out=x_sb[:], in_=e1c[:],
                        func=mybir.ActivationFunctionType.Ln,
                        bias=1.0,
                    )
                    cst[q]["x"] = x_sb

                def stage_mlp2(p):
                    d = st[p]
                    x_sb = cst[p // 2]["x"]
                    off = (p % 2) * 1024
                    for h in range(2):
                        nc.tensor.matmul(
                            out=d["xw"][:, h * 512 : (h + 1) * 512], lhsT=w2t_sb[:],
                            rhs=x_sb[:, off + h * 512 : off + (h + 1) * 512],
                            start=True, stop=True,
                        )

                def stage_acts2(p):
                    d = st[p]
                    e2_sb = pw.tile([P, 1024], dt.float32, tag="e2")
                    nc.scalar.activation(
                        out=e2_sb[:], in_=d["xw"][:],
                        func=mybir.ActivationFunctionType.Exp,
                        bias=b2p_sb[:, 0:1],
                    )
                    w2f_sb = pw.tile([P, 1024], dt.bfloat16, tag="w2f")
                    nc.scalar.activation(
                        out=w2f_sb[:], in_=e2_sb[:],
                        func=mybir.ActivationFunctionType.Ln,
                        bias=half_sb[:, 0:1],
                        scale=0.5,
                    )
                    d["w2f"] = w2f_sb

                def stage_mult(p):
                    d = st[p]
                    msgT_sb = pg.tile([P, 1024], dt.bfloat16, tag="msgT")
                    for h in range(2):
                        nc.vector.tensor_tensor(
                            out=msgT_sb[:, h * 512 : (h + 1) * 512],
                            in0=d["w2f"][:, h * 512 : (h + 1) * 512],
                            in1=d["h1g"][h][:],
                            op=mybir.AluOpType.mult,
                        )
                    d["msgT"] = msgT_sb

                def stage_msg_scatter(p):
                    # transpose 4 chunks at a time through the shared
                    # single-shot bank, copy to SBUF, scatter immediately
                    d = st[p]
                    msk = mo_ps[:, 0:256].bitcast(dt.bfloat16)  # [P, 512]
                    for h in range(2):
                        for t in range(4):
                            nc.tensor.transpose(
                                out=msk[:, t * P : (t + 1) * P],
                                in_=d["msgT"][:, (4 * h + t) * P : (4 * h + t + 1) * P],
                                identity=ident_sb[:],
                            )
                        msg_sb = pg.tile([P, 4, P], dt.bfloat16, tag="msg_sb")
                        nc.vector.tensor_copy(
                            out=msg_sb[:].rearrange("p t c -> p (t c)"),
                            in_=msk[:],
                        )
                        for t in range(4):
                            k = 8 * p + 4 * h + t
                            if k >= n_ch:
                                continue
                            b, first, last = sched[k]
                            if first:
                                # seed the accumulation with the block's own
                                # h1 rows: h2 = h1_own + scatter-sum in PSUM.
                                # agg regions alternate with block parity so
                                # a block never waits on the previous one's
                                # epilogue copy.
                                agg_state[0] = ao_ps[:, (b % 2) * P : (b % 2 + 1) * P]
                                nc.tensor.matmul(
                                    out=agg_state[0],
                                    lhsT=hown_sb[:, b * P : (b + 1) * P],
                                    rhs=lin1wt_sb[:],
                                    start=True, stop=False,
                                )
                            nc.tensor.matmul(
                                out=agg_state[0],
                                lhsT=d["s"][:, (4 * h + t) * P : (4 * h + t + 1) * P],
                                rhs=msg_sb[:, t, :],
                                start=False, stop=last,
                            )
                            if last:
                                # free the agg region now; the epilogue tail
                                # is flushed at the end of the loop body so
                                # it never head-blocks the PE queue
                                h2_sb = pep.tile([P, CH], dt.bfloat16, tag="h2")
                                nc.vector.tensor_copy(
                                    out=h2_sb[:], in_=agg_state[0]
                                )
                                pending.append((b, h2_sb))

                def stage_epilogue():
                    while pending:
                        b, h2_sb = pending.pop(0)
                        h2t_ap = mo_ps[:, 384:448].bitcast(dt.bfloat16)
                        nc.tensor.transpose(
                            out=h2t_ap, in_=h2_sb[:], identity=ident_sb[:]
                        )
                        h2T_sb = pep.tile([P, CH], dt.bfloat16, tag="h2T")
                        nc.vector.tensor_copy(out=h2T_sb[:], in_=h2t_ap)
                        o_ap = mo_ps[:, 256:384]
                        nc.tensor.matmul(
                            out=o_ap, lhsT=lin2wt_sb[:], rhs=h2T_sb[:],
                            start=True, stop=True,
                        )
                        o_sb = pep.tile([P, P], dt.float32, tag="o_sb")
                        nc.vector.tensor_scalar(
                            out=o_sb[:], in0=o_ap,
                            scalar1=l2b_sb[:, 0:1], scalar2=None,
                            op0=mybir.AluOpType.add,
                        )
                        nc.sync.dma_start(
                            out=out_t.ap()[:, b * P : (b + 1) * P], in_=o_sb[:]
                        )

                for p in range(n_pair + 2):
                    if p < n_pair:
                        stage_load(p)
                    if p == 0:
                        nc.sync.dma_start(out=w2t_sb[:], in_=w2t.ap())
                        nc.sync.dma_start(out=lin1wt_sb[:], in_=lin1wt.ap())
                        nc.sync.dma_start(out=lin2wt_sb[:], in_=lin2wt.ap())
                        nc.sync.dma_start(out=b2p_sb[:], in_=b2p.ap())
                        nc.sync.dma_start(out=l2b_sb[:], in_=l2b.ap())
                        nc.sync.dma_start(out=hown_sb[:], in_=hown_t.ap())
                    if p >= 2:
                        stage_acts2(p - 2)
                    if p < n_pair:
                        stage_mlp1(p)
                        stage_exp1(p)
                        if p % 2 == 1:
                            stage_ln1(p // 2)
                    if p >= 2:
                        stage_lin1(p - 2)
                        stage_mult(p - 2)
                        stage_msg_scatter(p - 2)
                        del st[p - 2]
                        if (p - 2) % 2 == 1:
                            cst.pop((p - 2) // 2, None)
                    if p < n_pair:
                        if p % 2 == 1:
                            stage_mlp2(p - 1)
                            stage_mlp2(p)
                    stage_epilogue()

    nc.compile()
    return nc


def prep_inputs(h, edge_index, edge_weight, edge_attr,
                lin1_w, nn_w1, nn_b1, nn_w2, nn_b2, lin2_w, lin2_b,
                n_nodes, ncores=NCORES):
    """Host-side sharding/layout. Returns (params, in_maps, meta)."""
    dst = np.asarray(edge_index[0], dtype=np.int64)
    src = np.asarray(edge_index[1], dtype=np.int64)
    ews = np.asarray(edge_weight, dtype=np.float32)
    eas = np.asarray(edge_attr, dtype=np.float32)
    cs = (0.5 * (np.cos(ews * (PI / CUTOFF)) + 1.0)).astype(np.float32)

    nblk_tot = _ceil_div(n_nodes, P)            # 391 real blocks
    nblk_slots = _ceil_div(nblk_tot, ncores) * ncores  # 392 incl. dummy
    nblk = nblk_slots // ncores                 # 49 positions per core

    blk = dst // P
    cnt = np.bincount(blk, minlength=nblk_slots)

    # deal blocks, sorted by count desc, round-robin to (position, core):
    # rank r -> position r // ncores on core r % ncores.  Every core's
    # position j then needs at most ceil(cnt[rank 8j] / 128) chunks.
    order_blocks = np.argsort(-cnt, kind="stable")
    k_list = []
    for j in range(nblk):
        k_list.append(max(1, int(_ceil_div(int(cnt[order_blocks[j * ncores]]), P))))
    n_ch = sum(k_list)
    n_chp = _ceil_div(n_ch, 16) * 16
    ne_pad = n_chp * P

    chunk_start = np.zeros(nblk + 1, dtype=np.int64)
    np.cumsum(np.asarray(k_list), out=chunk_start[1:])

    # per-edge rank within its block (stable order)
    order_e = np.argsort(blk, kind="stable")
    blk_sorted = blk[order_e]
    starts = np.searchsorted(blk_sorted, np.arange(nblk_slots))
    rank = np.empty(len(dst), dtype=np.int64)
    rank[order_e] = np.arange(len(dst), dtype=np.int64) - starts[blk_sorted]

    # block -> (core, position)
    pos_of_block = np.empty(nblk_slots, dtype=np.int64)
    core_of_block = np.empty(nblk_slots, dtype=np.int64)
    pos_of_block[order_blocks] = np.arange(nblk_slots) // ncores
    core_of_block[order_blocks] = np.arange(nblk_slots) % ncores

    ht = np.ascontiguousarray(np.asarray(h, dtype=np.float32).T)  # [CH, n]

    w1t_a = np.ascontiguousarray(np.asarray(nn_w1, np.float32).T).astype(BF16)
    w2t_a = np.ascontiguousarray(np.asarray(nn_w2, np.float32).T).astype(BF16)
    lin1wt_a = np.ascontiguousarray(np.asarray(lin1_w, np.float32).T).astype(BF16)
    lin2wt_a = np.ascontiguousarray(np.asarray(lin2_w, np.float32).T).astype(BF16)
    b1_a = np.asarray(nn_b1, np.float32).reshape(P, 1)
    b2p_a = (
        np.asarray(nn_b2, np.float64)
        - LOG2 * np.asarray(nn_w2, np.float64).sum(axis=1)
    ).astype(np.float32).reshape(P, 1)
    l2b_a = np.asarray(lin2_b, np.float32).reshape(P, 1)

    e_core = core_of_block[blk]
    e_pos = pos_of_block[blk]
    e_slot = (chunk_start[e_pos] + rank // P) * P + rank % P
    dstl = dst - blk * P

    in_maps = []
    blocks_of_core = []
    for c in range(ncores):
        m = e_core == c
        slot = e_slot[m]
        assert slot.max() < ne_pad

        he = np.zeros((P, ne_pad), dtype=BF16)
        he[:, slot] = ht[:, src[m]].astype(BF16)

        ea_pad = np.zeros((ne_pad, NG), dtype=BF16)
        ea_pad[slot] = eas[m].astype(BF16)

        s_all = np.zeros((P, ne_pad), dtype=BF16)
        s_all[slot % P, (slot // P) * P + dstl[m]] = cs[m].astype(BF16)

        # own blocks' h, position-major
        myblocks = order_blocks[np.arange(nblk) * ncores + c]
        hown = np.zeros((P, nblk * P), dtype=BF16)
        for j, b in enumerate(myblocks):
            lo = int(b) * P
            hi = min(lo + P, n_nodes)
            if lo < n_nodes:
                hown[:, j * P : j * P + (hi - lo)] = ht[:, lo:hi].astype(BF16)
        blocks_of_core.append(myblocks)

        in_maps.append({
            "he_t": he,
            "ea_t": np.ascontiguousarray(ea_pad.T),
            "s_t": s_all,
            "hown_t": hown,
            "w1t": w1t_a,
            "w2t": w2t_a,
            "lin1wt": lin1wt_a,
            "lin2wt": lin2wt_a,
            "b1": b1_a,
            "b2p": b2p_a,
            "l2b": l2b_a,
        })

    params = dict(n_chp=n_chp, k_list=tuple(k_list), nblk=nblk)
    meta = dict(n_nodes=n_nodes, ncores=ncores, nblk=nblk,
                blocks_of_core=blocks_of_core)
    return params, in_maps, meta


def assemble_output(results, meta):
    n_nodes = meta["n_nodes"]
    nblk = meta["nblk"]
    out = np.empty((n_nodes, CH), dtype=np.float32)
    for c in range(meta["ncores"]):
        o = results[c]["out_t"]  # [CH, nblk*P]
        for j, b in enumerate(meta["blocks_of_core"][c]):
            lo = int(b) * P
            hi = min(lo + P, n_nodes)
            if lo < n_nodes:
                out[lo:hi] = o[:, j * P : j * P + (hi - lo)].T
    return out


def kernel(**inputs):
    params, in_maps, meta = prep_inputs(
        inputs["h"], inputs["edge_index"], inputs["edge_weight"],
        inputs["edge_attr"], inputs["lin1_w"], inputs["nn_w1"],
        inputs["nn_b1"], inputs["nn_w2"], inputs["nn_b2"],
        inputs["lin2_w"], inputs["lin2_b"], N_NODES,
    )
    nc = build_program(**params)

    last_err = None
    for _attempt in range(3):
        try:
            br = bass_utils.run_bass_kernel_spmd(
                nc, in_maps, core_ids=list(range(NCORES))
            )
        except Exception as e:  # transient device errors: retry
            last_err = e
            continue
        return assemble_output(br.results, meta)
    raise last_err


# revision 28
# speedup vs baseline: 1.2001x; 1.2001x over previous
"""CFConv (SchNet continuous-filter convolution) on 8 TRN2 NeuronCores.

Strategy (v4, ~448us vs 1226us v1 baseline): the per-edge source gather is
done on the HOST as a pure permutation -- h rows are staged into edge order
(feature-major bf16) and streamed sequentially, so the device never runs a
dynamic gather (the v1 GpSimd SWDGE gather was 86% of the kernel).  lin1 is
applied per edge on the TensorEngine.  Destination blocks of 128 nodes are
load-balanced across the 8 cores by sorting all 392 blocks by edge count
and dealing rank 8j..8j+7 to position j of cores 0..7, so one SPMD program
(per-position chunk counts k_list) fits every core with ~4% padding.  The
scatter-add is a sequence of one-hot matmuls accumulating in PSUM; each
block's group STARTS with the lin1 matmul of the block's own nodes, so
h2 = h1 + agg falls out of PSUM for free.  The filter MLP runs
feature-major with stationary weights at 1024-edge width; softplus is Exp
then Ln(x+1) from the single exp+ln activation table (softplus-LOG2 via
Ln(0.5x+0.5)); the cosine cutoff is folded into the one-hot scatter values
on the host exactly as the reference factorizes it.

The loop is software-pipelined over 1024-edge pairs with a 2-pair skew
between the layer-1 acts of pair p and the layer-2 acts of pair p-2, which
keeps the scalar engine (the bottleneck: 4 activation passes per edge, its
streaming floor) >90% busy with zero-gap issue.  PSUM discipline: a bank
must never host two concurrently-open matmul accumulation groups, so the
two alternating agg accumulators (long-open groups) own one bank, all
single-shot outputs (transposed messages, lin2 out, transposed h2) share
another, and the deferred per-block epilogue is flushed at loop tails so
it never head-blocks the PE queue.
"""
import sys

sys.path.insert(0, "/opt/trn_rl_repo")

import numpy as np
import ml_dtypes

import concourse.bass as bass
import concourse.mybir as mybir
import concourse.tile as tile
from concourse import bacc
from concourse import bass_utils
from concourse import hw_specs
import concourse.bacc as bacc_mod
from concourse.masks import make_identity

BF16 = ml_dtypes.bfloat16
F32 = np.float32
LOG2 = float(np.log(2.0))
CUTOFF = 10.0
PI = float(np.pi)

N_NODES = 50000
N_EDGES = 800000
CH = 128
NG = 50
NCORES = 8
P = 128

dt = mybir.dt

# Route Exp/Ln to the single table that holds both, so the scalar engine
# never reloads activation tables mid-kernel.  Table ids are positional, so
# preserve dict order and only edit membership.
_orig_tables = hw_specs.get_activation_tables


def _patched_tables(arch):
    t = _orig_tables(arch)
    for name, funcs in t.items():
        if name != "natural_log_exp_and_others":
            funcs.discard(mybir.ActivationFunctionType.Exp)
            funcs.discard(mybir.ActivationFunctionType.Ln)
    return t


bacc_mod.get_activation_tables = _patched_tables


def _ceil_div(a, b):
    return -(-a // b)


def build_program(n_chp, k_list, nblk, num_devices=NCORES):
    nc = bacc.Bacc(
        "TRN2",
        target_bir_lowering=False,
        debug=False,
        enable_asserts=False,
        num_devices=num_devices,
    )

    ne_pad = n_chp * P
    n_sup = n_chp // 4
    n_ch = sum(k_list)

    # chunk -> (block position, first?, last?)
    sched = []
    for j, kb in enumerate(k_list):
        for i in range(kb):
            sched.append((j, i == 0, i == kb - 1))
    assert len(sched) == n_ch

    # ---- DRAM I/O ----
    he_t = nc.dram_tensor("he_t", [P, ne_pad], dt.bfloat16, kind="ExternalInput")
    ea_t = nc.dram_tensor("ea_t", [NG, ne_pad], dt.bfloat16, kind="ExternalInput")
    s_t = nc.dram_tensor("s_t", [P, ne_pad], dt.bfloat16, kind="ExternalInput")
    hown_t = nc.dram_tensor("hown_t", [P, nblk * P], dt.bfloat16, kind="ExternalInput")
    w1t = nc.dram_tensor("w1t", [NG, CH], dt.bfloat16, kind="ExternalInput")
    w2t = nc.dram_tensor("w2t", [CH, CH], dt.bfloat16, kind="ExternalInput")
    lin1wt = nc.dram_tensor("lin1wt", [CH, CH], dt.bfloat16, kind="ExternalInput")
    lin2wt = nc.dram_tensor("lin2wt", [CH, CH], dt.bfloat16, kind="ExternalInput")
    b1 = nc.dram_tensor("b1", [P, 1], dt.float32, kind="ExternalInput")
    b2p = nc.dram_tensor("b2p", [P, 1], dt.float32, kind="ExternalInput")
    l2b = nc.dram_tensor("l2b", [P, 1], dt.float32, kind="ExternalInput")

    out_t = nc.dram_tensor("out_t", [P, nblk * P], dt.float32, kind="ExternalOutput")

    with tile.TileContext(nc) as tc:
        with tc.tile_pool(name="cpool", bufs=1) as cpool:
            # ---- constants ----
            w1t_sb = cpool.tile([NG, CH], dt.bfloat16, tag="w1t")
            nc.sync.dma_start(out=w1t_sb[:], in_=w1t.ap())
            b1_sb = cpool.tile([P, 1], dt.float32, tag="b1")
            nc.sync.dma_start(out=b1_sb[:], in_=b1.ap())
            # remaining constants are loaded after the first stream loads
            # (below) so the first MLP pair starts as early as possible
            w2t_sb = cpool.tile([CH, CH], dt.bfloat16, tag="w2t")
            lin1wt_sb = cpool.tile([CH, CH], dt.bfloat16, tag="lin1wt")
            lin2wt_sb = cpool.tile([CH, CH], dt.bfloat16, tag="lin2wt")
            b2p_sb = cpool.tile([P, 1], dt.float32, tag="b2p")
            l2b_sb = cpool.tile([P, 1], dt.float32, tag="l2b")
            half_sb = cpool.tile([P, 1], dt.float32, tag="half")
            nc.gpsimd.memset(half_sb[:], 0.5)
            ident_sb = cpool.tile([P, P], dt.bfloat16, tag="ident")
            make_identity(nc, ident_sb[:])
            hown_sb = cpool.tile([P, nblk * P], dt.bfloat16, tag="hown")
            # (its DMA is issued after the first stream loads, below, so the
            # first MLP pair is not stuck behind 1.6 MB of skip-connection
            # data on the sync queue)

            # ---- edge pipeline (software-pipelined over 1024-edge pairs)
            # loop p issues: loads(p), acts-L2(p-2), mlp1(p), acts-L1(p),
            # lin1(p-2), mult(p-2), transpose+scatter(p-2), mlp2(p),
            # deferred epilogues.  The scalar stream is then gap-free and
            # the PE never head-blocks on a fresh dependency.
            with (
                tc.tile_pool(name="pea", bufs=4) as pea,
                tc.tile_pool(name="pst", bufs=4) as pst,
                tc.tile_pool(name="phe", bufs=4) as phe,
                tc.tile_pool(name="px", bufs=3) as px,
                tc.tile_pool(name="pw", bufs=3) as pw,
                tc.tile_pool(name="pg", bufs=3) as pg,
                tc.tile_pool(name="pep", bufs=3) as pep,
                tc.tile_pool(name="psxw", bufs=2, space="PSUM") as psxw,
                tc.tile_pool(name="psh", bufs=2, space="PSUM") as psh,
                tc.tile_pool(name="psm", bufs=1, space="PSUM") as psm,
                tc.tile_pool(name="psao", bufs=1, space="PSUM") as psao,
            ):
                n_pair = n_chp // 8
                st = {}       # per-pair live tiles
                agg_state = [None]  # current agg region across scatter calls
                pending = []  # deferred block epilogues

                # Persistent PSUM banks.  A bank must never host two
                # concurrently-open matmul accumulation groups, so the two
                # alternating agg accumulators (long-lived groups) get a bank
                # to themselves, and everything single-shot (transposed
                # messages, lin2 output, transposed h2) shares the other.
                ao_ps = psao.tile([P, 256], dt.float32, tag="aggo")
                mo_ps = psm.tile([P, 448], dt.float32, tag="mo")

                def stage_load(p):
                    es = p * 1024
                    ea_sb = pea.tile([NG, 1024], dt.bfloat16, tag="ea")
                    nc.sync.dma_start(out=ea_sb[:], in_=ea_t.ap()[:, es : es + 1024])
                    s_sb = pst.tile([P, 1024], dt.bfloat16, tag="s_sel")
                    nc.sync.dma_start(out=s_sb[:], in_=s_t.ap()[:, es : es + 1024])
                    he_sb = phe.tile([P, 1024], dt.bfloat16, tag="he")
                    nc.sync.dma_start(out=he_sb[:], in_=he_t.ap()[:, es : es + 1024])
                    st[p] = dict(ea=ea_sb, s=s_sb, he=he_sb)

                def stage_lin1(p):
                    d = st[p]
                    d["h1g"] = []
                    for h in range(2):
                        h1g_ps = psh.tile([P, 512], dt.float32, tag="h1g")
                        nc.tensor.matmul(
                            out=h1g_ps[:], lhsT=lin1wt_sb[:],
                            rhs=d["he"][:, h * 512 : (h + 1) * 512],
                            start=True, stop=True,
                        )
                        d["h1g"].append(h1g_ps)

                def stage_mlp1(p):
                    d = st[p]
                    xw_ps = psxw.tile([P, 1024], dt.float32, tag="xw_ps")
                    for h in range(2):
                        nc.tensor.matmul(
                            out=xw_ps[:, h * 512 : (h + 1) * 512], lhsT=w1t_sb[:],
                            rhs=d["ea"][:, h * 512 : (h + 1) * 512],
                            start=True, stop=True,
                        )
                    d["xw"] = xw_ps

                def stage_acts1(p):
                    d = st[p]
                    e1_sb = px.tile([P, 1024], dt.float32, tag="e1")
                    nc.scalar.activation(
                        out=e1_sb[:], in_=d["xw"][:],
                        func=mybir.ActivationFunctionType.Exp,
                        bias=b1_sb[:, 0:1],
                    )
                    x_sb = px.tile([P, 1024], dt.bfloat16, tag="x_sb")
                    nc.scalar.activation(
                        out=x_sb[:], in_=e1_sb[:],
                        func=mybir.ActivationFunctionType.Ln,
                        bias=1.0,
                    )
                    d["x"] = x_sb

                def stage_mlp2(p):
                    d = st[p]
                    for h in range(2):
                        nc.tensor.matmul(
                            out=d["xw"][:, h * 512 : (h + 1) * 512], lhsT=w2t_sb[:],
                            rhs=d["x"][:, h * 512 : (h + 1) * 512],
                            start=True, stop=True,
                        )

                def stage_acts2(p):
                    d = st[p]
                    e2_sb = pw.tile([P, 1024], dt.float32, tag="e2")
                    nc.scalar.activation(
                        out=e2_sb[:], in_=d["xw"][:],
                        func=mybir.ActivationFunctionType.Exp,
                        bias=b2p_sb[:, 0:1],
                    )
                    w2f_sb = pw.tile([P, 1024], dt.bfloat16, tag="w2f")
                    nc.scalar.activation(
                        out=w2f_sb[:], in_=e2_sb[:],
                        func=mybir.ActivationFunctionType.Ln,
                        bias=half_sb[:, 0:1],
                        scale=0.5,
                    )
                    d["w2f"] = w2f_sb

                def stage_mult(p):
                    d = st[p]
                    msgT_sb = pg.tile([P, 1024], dt.bfloat16, tag="msgT")
                    for h in range(2):
                        nc.vector.tensor_tensor(
                            out=msgT_sb[:, h * 512 : (h + 1) * 512],
                            in0=d["w2f"][:, h * 512 : (h + 1) * 512],
                            in1=d["h1g"][h][:],
                            op=mybir.AluOpType.mult,
                        )
                    d["msgT"] = msgT_sb

                def stage_msg_scatter(p):
                    # transpose 4 chunks at a time through the shared
                    # single-shot bank, copy to SBUF, scatter immediately
                    d = st[p]
                    msk = mo_ps[:, 0:256].bitcast(dt.bfloat16)  # [P, 512]
                    for h in range(2):
                        for t in range(4):
                            nc.tensor.transpose(
                                out=msk[:, t * P : (t + 1) * P],
                                in_=d["msgT"][:, (4 * h + t) * P : (4 * h + t + 1) * P],
                                identity=ident_sb[:],
                            )
                        msg_sb = pg.tile([P, 4, P], dt.bfloat16, tag="msg_sb")
                        nc.vector.tensor_copy(
                            out=msg_sb[:].rearrange("p t c -> p (t c)"),
                            in_=msk[:],
                        )
                        for t in range(4):
                            k = 8 * p + 4 * h + t
                            if k >= n_ch:
                                continue
                            b, first, last = sched[k]
                            if first:
                                # seed the accumulation with the block's own
                                # h1 rows: h2 = h1_own + scatter-sum in PSUM.
                                # agg regions alternate with block parity so
                                # a block never waits on the previous one's
                                # epilogue copy.
                                agg_state[0] = ao_ps[:, (b % 2) * P : (b % 2 + 1) * P]
                                nc.tensor.matmul(
                                    out=agg_state[0],
                                    lhsT=hown_sb[:, b * P : (b + 1) * P],
                                    rhs=lin1wt_sb[:],
                                    start=True, stop=False,
                                )
                            nc.tensor.matmul(
                                out=agg_state[0],
                                lhsT=d["s"][:, (4 * h + t) * P : (4 * h + t + 1) * P],
                                rhs=msg_sb[:, t, :],
                                start=False, stop=last,
                            )
                            if last:
                                # free the agg region now; the epilogue tail
                                # is flushed at the end of the loop body so
                                # it never head-blocks the PE queue
                                h2_sb = pep.tile([P, CH], dt.bfloat16, tag="h2")
                                nc.vector.tensor_copy(
                                    out=h2_sb[:], in_=agg_state[0]
                                )
                                pending.append((b, h2_sb))

                def stage_epilogue():
                    while pending:
                        b, h2_sb = pending.pop(0)
                        h2t_ap = mo_ps[:, 384:448].bitcast(dt.bfloat16)
                        nc.tensor.transpose(
                            out=h2t_ap, in_=h2_sb[:], identity=ident_sb[:]
                        )
                        h2T_sb = pep.tile([P, CH], dt.bfloat16, tag="h2T")
                        nc.vector.tensor_copy(out=h2T_sb[:], in_=h2t_ap)
                        o_ap = mo_ps[:, 256:384]
                        nc.tensor.matmul(
                            out=o_ap, lhsT=lin2wt_sb[:], rhs=h2T_sb[:],
                            start=True, stop=True,
                        )
                        o_sb = pep.tile([P, P], dt.float32, tag="o_sb")
                        nc.vector.tensor_scalar(
                            out=o_sb[:], in0=o_ap,
                            scalar1=l2b_sb[:, 0:1], scalar2=None,
                            op0=mybir.AluOpType.add,
                        )
                        nc.sync.dma_start(
                            out=out_t.ap()[:, b * P : (b + 1) * P], in_=o_sb[:]
                        )

                for p in range(n_pair + 2):
                    if p < n_pair:
                        stage_load(p)
                    if p == 0:
                        nc.sync.dma_start(out=w2t_sb[:], in_=w2t.ap())
                        nc.sync.dma_start(out=lin1wt_sb[:], in_=lin1wt.ap())
                        nc.sync.dma_start(out=lin2wt_sb[:], in_=lin2wt.ap())
                        nc.sync.dma_start(out=b2p_sb[:], in_=b2p.ap())
                        nc.sync.dma_start(out=l2b_sb[:], in_=l2b.ap())
                        nc.sync.dma_start(out=hown_sb[:], in_=hown_t.ap())
                    if p >= 2:
                        stage_acts2(p - 2)
                    if p < n_pair:
                        stage_mlp1(p)
                        stage_acts1(p)
                    if p >= 2:
                        stage_lin1(p - 2)
                        stage_mult(p - 2)
                        stage_msg_scatter(p - 2)
                        del st[p - 2]
                    if p < n_pair:
                        stage_mlp2(p)
                    stage_epilogue()

    nc.compile()
    return nc


def prep_inputs(h, edge_index, edge_weight, edge_attr,
                lin1_w, nn_w1, nn_b1, nn_w2, nn_b2, lin2_w, lin2_b,
                n_nodes, ncores=NCORES):
    """Host-side sharding/layout. Returns (params, in_maps, meta)."""
    dst = np.asarray(edge_index[0], dtype=np.int64)
    src = np.asarray(edge_index[1], dtype=np.int64)
    ews = np.asarray(edge_weight, dtype=np.float32)
    eas = np.asarray(edge_attr, dtype=np.float32)
    cs = (0.5 * (np.cos(ews * (PI / CUTOFF)) + 1.0)).astype(np.float32)

    nblk_tot = _ceil_div(n_nodes, P)            # 391 real blocks
    nblk_slots = _ceil_div(nblk_tot, ncores) * ncores  # 392 incl. dummy
    nblk = nblk_slots // ncores                 # 49 positions per core

    blk = dst // P
    cnt = np.bincount(blk, minlength=nblk_slots)

    # deal blocks, sorted by count desc, round-robin to (position, core):
    # rank r -> position r // ncores on core r % ncores.  Every core's
    # position j then needs at most ceil(cnt[rank 8j] / 128) chunks.
    order_blocks = np.argsort(-cnt, kind="stable")
    k_list = []
    for j in range(nblk):
        k_list.append(max(1, int(_ceil_div(int(cnt[order_blocks[j * ncores]]), P))))
    n_ch = sum(k_list)
    n_chp = _ceil_div(n_ch, 8) * 8
    ne_pad = n_chp * P

    chunk_start = np.zeros(nblk + 1, dtype=np.int64)
    np.cumsum(np.asarray(k_list), out=chunk_start[1:])

    # per-edge rank within its block (stable order)
    order_e = np.argsort(blk, kind="stable")
    blk_sorted = blk[order_e]
    starts = np.searchsorted(blk_sorted, np.arange(nblk_slots))
    rank = np.empty(len(dst), dtype=np.int64)
    rank[order_e] = np.arange(len(dst), dtype=np.int64) - starts[blk_sorted]

    # block -> (core, position)
    pos_of_block = np.empty(nblk_slots, dtype=np.int64)
    core_of_block = np.empty(nblk_slots, dtype=np.int64)
    pos_of_block[order_blocks] = np.arange(nblk_slots) // ncores
    core_of_block[order_blocks] = np.arange(nblk_slots) % ncores

    ht = np.ascontiguousarray(np.asarray(h, dtype=np.float32).T)  # [CH, n]

    w1t_a = np.ascontiguousarray(np.asarray(nn_w1, np.float32).T).astype(BF16)
    w2t_a = np.ascontiguousarray(np.asarray(nn_w2, np.float32).T).astype(BF16)
    lin1wt_a = np.ascontiguousarray(np.asarray(lin1_w, np.float32).T).astype(BF16)
    lin2wt_a = np.ascontiguousarray(np.asarray(lin2_w, np.float32).T).astype(BF16)
    b1_a = np.asarray(nn_b1, np.float32).reshape(P, 1)
    b2p_a = (
        np.asarray(nn_b2, np.float64)
        - LOG2 * np.asarray(nn_w2, np.float64).sum(axis=1)
    ).astype(np.float32).reshape(P, 1)
    l2b_a = np.asarray(lin2_b, np.float32).reshape(P, 1)

    e_core = core_of_block[blk]
    e_pos = pos_of_block[blk]
    e_slot = (chunk_start[e_pos] + rank // P) * P + rank % P
    dstl = dst - blk * P

    in_maps = []
    blocks_of_core = []
    for c in range(ncores):
        m = e_core == c
        slot = e_slot[m]
        assert slot.max() < ne_pad

        he = np.zeros((P, ne_pad), dtype=BF16)
        he[:, slot] = ht[:, src[m]].astype(BF16)

        ea_pad = np.zeros((ne_pad, NG), dtype=BF16)
        ea_pad[slot] = eas[m].astype(BF16)

        s_all = np.zeros((P, ne_pad), dtype=BF16)
        s_all[slot % P, (slot // P) * P + dstl[m]] = cs[m].astype(BF16)

        # own blocks' h, position-major
        myblocks = order_blocks[np.arange(nblk) * ncores + c]
        hown = np.zeros((P, nblk * P), dtype=BF16)
        for j, b in enumerate(myblocks):
            lo = int(b) * P
            hi = min(lo + P, n_nodes)
            if lo < n_nodes:
                hown[:, j * P : j * P + (hi - lo)] = ht[:, lo:hi].astype(BF16)
        blocks_of_core.append(myblocks)

        in_maps.append({
            "he_t": he,
            "ea_t": np.ascontiguousarray(ea_pad.T),
            "s_t": s_all,
            "hown_t": hown,
            "w1t": w1t_a,
            "w2t": w2t_a,
            "lin1wt": lin1wt_a,
            "lin2wt": lin2wt_a,
            "b1": b1_a,
            "b2p": b2p_a,
            "l2b": l2b_a,
        })

    params = dict(n_chp=n_chp, k_list=tuple(k_list), nblk=nblk)
    meta = dict(n_nodes=n_nodes, ncores=ncores, nblk=nblk,
                blocks_of_core=blocks_of_core)
    return params, in_maps, meta


def assemble_output(results, meta):
    n_nodes = meta["n_nodes"]
    nblk = meta["nblk"]
    out = np.empty((n_nodes, CH), dtype=np.float32)
    for c in range(meta["ncores"]):
        o = results[c]["out_t"]  # [CH, nblk*P]
        for j, b in enumerate(meta["blocks_of_core"][c]):
            lo = int(b) * P
            hi = min(lo + P, n_nodes)
            if lo < n_nodes:
                out[lo:hi] = o[:, j * P : j * P + (hi - lo)].T
    return out


def kernel(**inputs):
    params, in_maps, meta = prep_inputs(
        inputs["h"], inputs["edge_index"], inputs["edge_weight"],
        inputs["edge_attr"], inputs["lin1_w"], inputs["nn_w1"],
        inputs["nn_b1"], inputs["nn_w2"], inputs["nn_b2"],
        inputs["lin2_w"], inputs["lin2_b"], N_NODES,
    )
    nc = build_program(**params)

    last_err = None
    for _attempt in range(3):
        try:
            br = bass_utils.run_bass_kernel_spmd(
                nc, in_maps, core_ids=list(range(NCORES))
            )
        except Exception as e:  # transient device errors: retry
            last_err = e
            continue
        return assemble_output(br.results, meta)
    raise last_err
